# revision 4
# baseline (speedup 1.0000x reference)
"""Trainium2 Bass kernel for nn_CrossAttentionBlock.

Math note: the reference's attention has a length-1 key axis, so
softmax(attn, axis=-1) == 1.0 exactly and the attention output equals v
broadcast over the HW query axis.  The GroupNorm -> Wq -> q@k path is
therefore mathematically dead.  The exact output is

    out[b, c, h, w] = x[b, c, h, w] + y[b, c]
    y[b]            = Wout @ v[b] + bout
    v[b]            = Wkv[C:2C, :] @ context[b] + bkv[C:2C]

Sharding: pure data parallel over batch B=32 -> 4 batches per core on 8
cores; the small weights are replicated (passed pre-transposed so the
TensorEngine can consume them as lhsT without on-device transposes).
Per core the kernel computes the tiny matmuls on the TensorEngine and
then streams the 16.8 MB x-shard through SBUF adding the per-(b,c)
scalar: the kernel is HBM-bandwidth-bound.
"""

import numpy as np

import concourse.bass as bass
import concourse.mybir as mybir
import concourse.tile as tile
from concourse import bacc
from concourse.bass_utils import run_bass_kernel_spmd

N_CORES = 8
B = 32
C = 256
HW = 64 * 64
CTX = 512
B_LOC = B // N_CORES                # 4 batches per core
ROWS = B_LOC * C                    # 1024 rows of the flattened x-shard
ROW_TILES = ROWS // 128             # 8 streaming tiles of [128, HW]
FP32 = mybir.dt.float32

_cache: dict = {}


def _build_nc() -> bass.Bass:
    nc = bacc.Bacc("TRN2", target_bir_lowering=False, debug=False)

    xs = nc.dram_tensor("xs", [ROWS, HW], FP32, kind="ExternalInput")
    # ctxT = context_shard.T [CTX, B_LOC]; wkvT = Wkv[C:2C].T [CTX, C];
    # woT = Wout.T [C, C] - transposed on host (layout prep of the small
    # replicated weights) so they load directly as matmul lhsT/rhs.
    ctxT = nc.dram_tensor("ctxT", [CTX, B_LOC], FP32, kind="ExternalInput")
    wkvT = nc.dram_tensor("wkvT", [CTX, C], FP32, kind="ExternalInput")
    bkv_v = nc.dram_tensor("bkv_v", [C], FP32, kind="ExternalInput")
    woT = nc.dram_tensor("woT", [C, C], FP32, kind="ExternalInput")
    bout = nc.dram_tensor("bout", [C], FP32, kind="ExternalInput")
    out = nc.dram_tensor("out", [ROWS, HW], FP32, kind="ExternalOutput")

    KC = CTX // 128                 # 4 k-chunks
    CC = C // 128                   # 2 c-chunks

    with tile.TileContext(nc) as tc:
        with (
            tc.tile_pool(name="consts", bufs=1) as consts,
            tc.tile_pool(name="psum_mm", bufs=2, space="PSUM") as psum_mm,
            tc.tile_pool(name="stream", bufs=8) as stream,
        ):
            # ---- streaming x loads first: no deps, start immediately ----
            xtiles = []
            for t in range(ROW_TILES):
                xt = stream.tile([128, HW], FP32)
                nc.sync.dma_start(out=xt[:], in_=xs[t * 128:(t + 1) * 128, :])
                xtiles.append(xt)

            # ---- small-weight pipeline on the gpsimd DMA ring ----
            ctxT_sb = consts.tile([128, KC, B_LOC], FP32)
            nc.gpsimd.dma_start(out=ctxT_sb[:], in_=ctxT.rearrange("(o p) b -> p o b", p=128))
            wkvT_sb = consts.tile([128, KC, C], FP32)
            nc.gpsimd.dma_start(out=wkvT_sb[:], in_=wkvT.rearrange("(o p) c -> p o c", p=128))
            woT_sb = consts.tile([128, CC, C], FP32)
            nc.gpsimd.dma_start(out=woT_sb[:], in_=woT.rearrange("(o p) c -> p o c", p=128))
            bkv_sb = consts.tile([128, CC], FP32)
            nc.gpsimd.dma_start(out=bkv_sb[:], in_=bkv_v.rearrange("(o p) -> p o", p=128))
            bout_sb = consts.tile([128, CC], FP32)
            nc.gpsimd.dma_start(out=bout_sb[:], in_=bout.rearrange("(o p) -> p o", p=128))

            # v[c, b] = Wkv_v @ ctx^T + bkv_v
            v_sb = consts.tile([128, CC, B_LOC], FP32)
            for cc in range(CC):
                pv = psum_mm.tile([128, B_LOC], FP32)
                for kc in range(KC):
                    nc.tensor.matmul(
                        pv[:],
                        wkvT_sb[:, kc, cc * 128:(cc + 1) * 128],
                        ctxT_sb[:, kc, :],
                        start=(kc == 0),
                        stop=(kc == KC - 1),
                    )
                nc.vector.tensor_tensor(
                    v_sb[:, cc, :],
                    pv[:],
                    bkv_sb[:, cc:cc + 1].to_broadcast([128, B_LOC]),
                    mybir.AluOpType.add,
                )

            # y[o, b] = Wout @ v + bout, laid out as yb[p, oc, b]
            yb = consts.tile([128, CC, B_LOC], FP32)
            for oc in range(CC):
                py = psum_mm.tile([128, B_LOC], FP32)
                for cc in range(CC):
                    nc.tensor.matmul(
                        py[:],
                        woT_sb[:, cc, oc * 128:(oc + 1) * 128],
                        v_sb[:, cc, :],
                        start=(cc == 0),
                        stop=(cc == CC - 1),
                    )
                nc.vector.tensor_tensor(
                    yb[:, oc, :],
                    py[:],
                    bout_sb[:, oc:oc + 1].to_broadcast([128, B_LOC]),
                    mybir.AluOpType.add,
                )

            # ---- add + store: out = x + y[b, c] ----
            for t in range(ROW_TILES):
                b, oc = t // CC, t % CC
                xt = xtiles[t]
                nc.vector.tensor_tensor(
                    xt[:],
                    xt[:],
                    yb[:, oc, b:b + 1].to_broadcast([128, HW]),
                    mybir.AluOpType.add,
                )
                # stores on the ACT HWDGE ring so they don't queue behind loads
                nc.scalar.dma_start(out=out[t * 128:(t + 1) * 128, :], in_=xt[:])

    nc.finalize()
    return nc


def kernel(x, context, gn_w=None, gn_b=None, Wq=None, bq=None, Wkv=None,
           bkv=None, Wout=None, bout=None, _trace=False):
    # gn_w/gn_b/Wq/bq and the k-half of Wkv/bkv are mathematically dead
    # (softmax over a length-1 axis is exactly 1), so they are unused.
    x = np.ascontiguousarray(np.asarray(x, dtype=np.float32))
    context = np.ascontiguousarray(np.asarray(context, dtype=np.float32))
    Wkv = np.asarray(Wkv, dtype=np.float32)
    bkv = np.asarray(bkv, dtype=np.float32)
    wkvT = np.ascontiguousarray(Wkv[C:2 * C].T)
    bkv_v = np.ascontiguousarray(bkv[C:2 * C])
    woT = np.ascontiguousarray(np.asarray(Wout, dtype=np.float32).T)
    bout_np = np.ascontiguousarray(np.asarray(bout, dtype=np.float32))

    if "nc" not in _cache:
        _cache["nc"] = _build_nc()
    nc = _cache["nc"]

    in_maps = []
    for c in range(N_CORES):
        xs = x[c * B_LOC:(c + 1) * B_LOC].reshape(ROWS, HW)
        in_maps.append({
            "xs": np.ascontiguousarray(xs),
            "ctxT": np.ascontiguousarray(context[c * B_LOC:(c + 1) * B_LOC].T),
            "wkvT": wkvT,
            "bkv_v": bkv_v,
            "woT": woT,
            "bout": bout_np,
        })

    res = run_bass_kernel_spmd(nc, in_maps, core_ids=list(range(N_CORES)),
                               trace=_trace)
    kernel.last_result = res
    out = np.concatenate(
        [r["out"].reshape(B_LOC, C, 64, 64) for r in res.results], axis=0
    )
    return out


# revision 7
# speedup vs baseline: 1.0084x; 1.0084x over previous
"""Trainium2 Bass kernel for nn_CrossAttentionBlock.

Math note: the reference's attention has a length-1 key axis, so
softmax(attn, axis=-1) == 1.0 exactly and the attention output equals v
broadcast over the HW query axis.  The GroupNorm -> Wq -> q@k path is
therefore mathematically dead.  The exact output is

    out[b, c, h, w] = x[b, c, h, w] + y[b, c]
    y[b]            = Wout @ v[b] + bout
    v[b]            = Wkv[C:2C, :] @ context[b] + bkv[C:2C]

Sharding: pure data parallel over batch B=32 -> 4 batches per core on 8
cores; the small weights are replicated (passed pre-transposed so the
TensorEngine can consume them as lhsT without on-device transposes).
Per core the kernel computes the tiny matmuls on the TensorEngine and
then streams the 16.8 MB x-shard through SBUF adding the per-(b,c)
scalar: the kernel is HBM-bandwidth-bound.
"""

import numpy as np

import concourse.bass as bass
import concourse.mybir as mybir
import concourse.tile as tile
from concourse import bacc
from concourse.bass_utils import run_bass_kernel_spmd

N_CORES = 8
B = 32
C = 256
HW = 64 * 64
CTX = 512
B_LOC = B // N_CORES                # 4 batches per core
ROWS = B_LOC * C                    # 1024 rows of the flattened x-shard
ROW_TILES = ROWS // 128             # 8 row-chunks of 128 rows
COL_SPLIT = 2                       # split HW into 1MB tiles for pipelining
COLS = HW // COL_SPLIT
FP32 = mybir.dt.float32

_cache: dict = {}


def _build_nc() -> bass.Bass:
    nc = bacc.Bacc("TRN2", target_bir_lowering=False, debug=False)

    xs = nc.dram_tensor("xs", [ROWS, HW], FP32, kind="ExternalInput")
    # ctxT = context_shard.T [CTX, B_LOC]; wkvT = Wkv[C:2C].T [CTX, C];
    # woT = Wout.T [C, C] - transposed on host (layout prep of the small
    # replicated weights) so they load directly as matmul lhsT/rhs.
    ctxT = nc.dram_tensor("ctxT", [CTX, B_LOC], FP32, kind="ExternalInput")
    wkvT = nc.dram_tensor("wkvT", [CTX, C], FP32, kind="ExternalInput")
    bkv_v = nc.dram_tensor("bkv_v", [C], FP32, kind="ExternalInput")
    woT = nc.dram_tensor("woT", [C, C], FP32, kind="ExternalInput")
    bout = nc.dram_tensor("bout", [C], FP32, kind="ExternalInput")
    out = nc.dram_tensor("out", [ROWS, HW], FP32, kind="ExternalOutput")

    KC = CTX // 128                 # 4 k-chunks
    CC = C // 128                   # 2 c-chunks

    with tile.TileContext(nc) as tc:
        with (
            tc.tile_pool(name="consts", bufs=1) as consts,
            tc.tile_pool(name="psum_mm", bufs=2, space="PSUM") as psum_mm,
            tc.tile_pool(name="stream", bufs=12) as stream,
        ):
            # ---- small-weight loads first on the sync HWDGE ring (fast,
            # ~1us total) so yb is ready by the time the first x tile lands
            ctxT_sb = consts.tile([128, KC, B_LOC], FP32)
            nc.sync.dma_start(out=ctxT_sb[:], in_=ctxT.rearrange("(o p) b -> p o b", p=128))
            wkvT_sb = consts.tile([128, KC, C], FP32)
            nc.sync.dma_start(out=wkvT_sb[:], in_=wkvT.rearrange("(o p) c -> p o c", p=128))
            woT_sb = consts.tile([128, CC, C], FP32)
            nc.sync.dma_start(out=woT_sb[:], in_=woT.rearrange("(o p) c -> p o c", p=128))
            bkv_sb = consts.tile([128, CC], FP32)
            nc.sync.dma_start(out=bkv_sb[:], in_=bkv_v.rearrange("(o p) -> p o", p=128))
            bout_sb = consts.tile([128, CC], FP32)
            nc.sync.dma_start(out=bout_sb[:], in_=bout.rearrange("(o p) -> p o", p=128))

            # v[c, b] = Wkv_v @ ctx^T + bkv_v
            v_sb = consts.tile([128, CC, B_LOC], FP32)
            for cc in range(CC):
                pv = psum_mm.tile([128, B_LOC], FP32)
                for kc in range(KC):
                    nc.tensor.matmul(
                        pv[:],
                        wkvT_sb[:, kc, cc * 128:(cc + 1) * 128],
                        ctxT_sb[:, kc, :],
                        start=(kc == 0),
                        stop=(kc == KC - 1),
                    )
                nc.vector.tensor_tensor(
                    v_sb[:, cc, :],
                    pv[:],
                    bkv_sb[:, cc:cc + 1].to_broadcast([128, B_LOC]),
                    mybir.AluOpType.add,
                )

            # y[o, b] = Wout @ v + bout, laid out as yb[p, oc, b]
            yb = consts.tile([128, CC, B_LOC], FP32)
            for oc in range(CC):
                py = psum_mm.tile([128, B_LOC], FP32)
                for cc in range(CC):
                    nc.tensor.matmul(
                        py[:],
                        woT_sb[:, cc, oc * 128:(oc + 1) * 128],
                        v_sb[:, cc, :],
                        start=(cc == 0),
                        stop=(cc == CC - 1),
                    )
                nc.vector.tensor_tensor(
                    yb[:, oc, :],
                    py[:],
                    bout_sb[:, oc:oc + 1].to_broadcast([128, B_LOC]),
                    mybir.AluOpType.add,
                )

            # ---- stream x: load (sync ring) -> add -> store (scalar ring)
            for t in range(ROW_TILES):
                b, oc = t // CC, t % CC
                for j in range(COL_SPLIT):
                    xt = stream.tile([128, COLS], FP32)
                    nc.sync.dma_start(
                        out=xt[:],
                        in_=xs[t * 128:(t + 1) * 128, j * COLS:(j + 1) * COLS],
                    )
                    nc.vector.tensor_tensor(
                        xt[:],
                        xt[:],
                        yb[:, oc, b:b + 1].to_broadcast([128, COLS]),
                        mybir.AluOpType.add,
                    )
                    nc.scalar.dma_start(
                        out=out[t * 128:(t + 1) * 128, j * COLS:(j + 1) * COLS],
                        in_=xt[:],
                    )

    nc.finalize()
    return nc


def kernel(x, context, gn_w=None, gn_b=None, Wq=None, bq=None, Wkv=None,
           bkv=None, Wout=None, bout=None, _trace=False):
    # gn_w/gn_b/Wq/bq and the k-half of Wkv/bkv are mathematically dead
    # (softmax over a length-1 axis is exactly 1), so they are unused.
    x = np.ascontiguousarray(np.asarray(x, dtype=np.float32))
    context = np.ascontiguousarray(np.asarray(context, dtype=np.float32))
    Wkv = np.asarray(Wkv, dtype=np.float32)
    bkv = np.asarray(bkv, dtype=np.float32)
    wkvT = np.ascontiguousarray(Wkv[C:2 * C].T)
    bkv_v = np.ascontiguousarray(bkv[C:2 * C])
    woT = np.ascontiguousarray(np.asarray(Wout, dtype=np.float32).T)
    bout_np = np.ascontiguousarray(np.asarray(bout, dtype=np.float32))

    if "nc" not in _cache:
        _cache["nc"] = _build_nc()
    nc = _cache["nc"]

    in_maps = []
    for c in range(N_CORES):
        xs = x[c * B_LOC:(c + 1) * B_LOC].reshape(ROWS, HW)
        in_maps.append({
            "xs": np.ascontiguousarray(xs),
            "ctxT": np.ascontiguousarray(context[c * B_LOC:(c + 1) * B_LOC].T),
            "wkvT": wkvT,
            "bkv_v": bkv_v,
            "woT": woT,
            "bout": bout_np,
        })

    res = run_bass_kernel_spmd(nc, in_maps, core_ids=list(range(N_CORES)),
                               trace=_trace)
    kernel.last_result = res
    out = np.concatenate(
        [r["out"].reshape(B_LOC, C, 64, 64) for r in res.results], axis=0
    )
    return out


# revision 8
# speedup vs baseline: 1.0295x; 1.0209x over previous
"""Trainium2 Bass kernel for nn_CrossAttentionBlock (raw Bass, no Tile).

Math note: the reference's attention has a length-1 key axis, so
softmax(attn, axis=-1) == 1.0 exactly and the attention output equals v
broadcast over the HW query axis.  The GroupNorm -> Wq -> q@k path is
therefore mathematically dead.  The exact output is

    out[b, c, h, w] = x[b, c, h, w] + y[b, c]
    y[b]            = Wout @ v[b] + bout
    v[b]            = Wkv[C:2C, :] @ context[b] + bkv[C:2C]

Sharding: pure data parallel over batch B=32 -> 4 batches per core on
8 cores; the small weights are replicated (passed pre-transposed so the
TensorEngine consumes them directly as matmul lhsT).  Per core the
kernel computes the tiny matmuls on the TensorEngine and streams the
16.8 MB x-shard through SBUF adding the per-(b,c) scalar — the kernel
is HBM-bandwidth-bound (~427 GB/s/core sustained on both DMA rings).

Raw engine programs with manual semaphores (no Tile/Bacc framework
barriers):
  sync   : x tile 0, the 5 small weight DMAs, x tiles 1-15 (HWDGE ring)
  tensor : 12 tiny matmuls (PSUM, one full bank per tile)
  vector : v/yb bias adds, then per-tile broadcast add (in place)
  scalar : per-tile store DMAs on the other HWDGE ring + final wait
All 16 x-tiles are SBUF-resident (no buffer reuse, no load gating).
"""

import numpy as np

import concourse.bass as bass
import concourse.mybir as mybir
from concourse.bass_utils import run_bass_kernel_spmd

N_CORES = 8
B = 32
C = 256
HW = 64 * 64
CTX = 512
B_LOC = B // N_CORES
ROWS = B_LOC * C                 # 1024
COLS = 2048                      # 1MB tiles [128, 2048]
N_TILES = (ROWS // 128) * (HW // COLS)   # 16
KC = CTX // 128                  # 4
CC = C // 128                    # 2
FP32 = mybir.dt.float32

_cache: dict = {}


def _build_nc() -> bass.Bass:
    nc = bass.Bass(target_bir_lowering=False)

    xs = nc.dram_tensor("xs", [ROWS, HW], FP32, kind="ExternalInput")
    ctxT = nc.dram_tensor("ctxT", [CTX, B_LOC], FP32, kind="ExternalInput")
    wkvT = nc.dram_tensor("wkvT", [CTX, C], FP32, kind="ExternalInput")
    # biases pre-striped on host to [128, CC] (partition p, chunk o)
    bkv_v = nc.dram_tensor("bkv_v", [128, CC], FP32, kind="ExternalInput")
    woT = nc.dram_tensor("woT", [C, C], FP32, kind="ExternalInput")
    bout = nc.dram_tensor("bout", [128, CC], FP32, kind="ExternalInput")
    out = nc.dram_tensor("out", [ROWS, HW], FP32, kind="ExternalOutput")

    def tile_src(idx):
        t, j = idx // 2, idx % 2
        return xs[t * 128:(t + 1) * 128, j * COLS:(j + 1) * COLS]

    def tile_dst(idx):
        t, j = idx // 2, idx % 2
        return out[t * 128:(t + 1) * 128, j * COLS:(j + 1) * COLS]

    def bias_col(idx):
        t = idx // 2
        return (t % CC) * B_LOC + t // CC   # column in yb [128, CC*B_LOC]

    xts = [nc.alloc_sbuf_tensor(f"xt{i}", [128, COLS], FP32) for i in range(N_TILES)]

    # one sem per load: with several DMAs in flight on one sem, the 16
    # per-SDMA-engine unit-increments can interleave across DMAs, so a
    # partial-progress wait (>= 16*(i+1)) would not imply tile i landed.
    # Dedicated sems make the per-tile wait exact; total-completion waits
    # (s_w >= 80, s_store >= 256) are safe on a shared sem.
    s_loads = [nc.alloc_semaphore(f"s_load{i}") for i in range(N_TILES)]

    with (
        nc.Block() as block,
        nc.semaphore("s_w") as s_w,
        nc.semaphore("s_mm") as s_mm,
        nc.semaphore("s_v") as s_v,
        nc.semaphore("s_add") as s_add,
        nc.semaphore("s_store") as s_store,
        nc.sbuf_tensor("wkvT_sb", [128, KC, C], FP32) as wkvT_sb,
        nc.sbuf_tensor("woT_sb", [128, CC, C], FP32) as woT_sb,
        nc.sbuf_tensor("ctxT_sb", [128, KC, B_LOC], FP32) as ctxT_sb,
        nc.sbuf_tensor("bkv_sb", [128, CC], FP32) as bkv_sb,
        nc.sbuf_tensor("bout_sb", [128, CC], FP32) as bout_sb,
        nc.sbuf_tensor("v_sb", [128, CC * B_LOC], FP32) as v_sb,
        nc.sbuf_tensor("yb", [128, CC * B_LOC], FP32) as yb,
        nc.psum_tensor("pv0", [128, 512], FP32) as pv0,
        nc.psum_tensor("pv1", [128, 512], FP32) as pv1,
        nc.psum_tensor("py0", [128, 512], FP32) as py0,
        nc.psum_tensor("py1", [128, 512], FP32) as py1,
    ):
        pv = [pv0, pv1]
        py = [py0, py1]

        @block.sync
        def _(sync):
            # first x tile first (its add gates the first store), then the
            # small weights, then the rest of the x tiles
            sync.dma_start(xts[0][:, :], tile_src(0)).then_inc(s_loads[0], 16)
            sync.dma_start(
                ctxT_sb[:, :, :], ctxT.rearrange("(o p) b -> p o b", p=128)
            ).then_inc(s_w, 16)
            sync.dma_start(
                wkvT_sb[:, :, :], wkvT.rearrange("(o p) c -> p o c", p=128)
            ).then_inc(s_w, 16)
            sync.dma_start(
                woT_sb[:, :, :], woT.rearrange("(s p) o -> p s o", p=128)
            ).then_inc(s_w, 16)
            sync.dma_start(bkv_sb[:, :], bkv_v[:, :]).then_inc(s_w, 16)
            sync.dma_start(bout_sb[:, :], bout[:, :]).then_inc(s_w, 16)
            for i in range(1, N_TILES):
                sync.dma_start(xts[i][:, :], tile_src(i)).then_inc(s_loads[i], 16)

        @block.tensor
        def _(tensor):
            tensor.wait_ge(s_w, 80)
            # v[c, b] = Wkv_v @ ctx^T  (2 c-chunks x 4 k-chunks)
            for cc in range(CC):
                for kc in range(KC):
                    nc.tensor.matmul(
                        pv[cc][:, :B_LOC],
                        wkvT_sb[:, kc, cc * 128:cc * 128 + 128],
                        ctxT_sb[:, kc, :],
                        start=(kc == 0),
                        stop=(kc == KC - 1),
                    )
                nc.tensor.drain().then_inc(s_mm, 1)
            # y[o, b] = Wout @ v  (needs v_sb from vector)
            tensor.wait_ge(s_v, 2)
            for oc in range(CC):
                for cc in range(CC):
                    nc.tensor.matmul(
                        py[oc][:, :B_LOC],
                        woT_sb[:, cc, oc * 128:oc * 128 + 128],
                        v_sb[:, cc * B_LOC:(cc + 1) * B_LOC],
                        start=(cc == 0),
                        stop=(cc == CC - 1),
                    )
                nc.tensor.drain().then_inc(s_mm, 1)

        @block.vector
        def _(vector):
            for cc in range(CC):
                vector.wait_ge(s_mm, cc + 1)
                nc.vector.tensor_tensor(
                    v_sb[:, cc * B_LOC:(cc + 1) * B_LOC],
                    pv[cc][:, :B_LOC],
                    bkv_sb[:, cc:cc + 1].to_broadcast([128, B_LOC]),
                    mybir.AluOpType.add,
                ).then_inc(s_v, 1)
            for oc in range(CC):
                vector.wait_ge(s_mm, CC + oc + 1)
                nc.vector.tensor_tensor(
                    yb[:, oc * B_LOC:(oc + 1) * B_LOC],
                    py[oc][:, :B_LOC],
                    bout_sb[:, oc:oc + 1].to_broadcast([128, B_LOC]),
                    mybir.AluOpType.add,
                )
            # drain the DVE pipeline: the tile adds read yb written above
            # on the same engine (deep pipeline, in-order but uncommitted)
            nc.vector.drain()
            for i in range(N_TILES):
                vector.wait_ge(s_loads[i], 16)
                c = bias_col(i)
                nc.vector.tensor_tensor(
                    xts[i][:, :],
                    xts[i][:, :],
                    yb[:, c:c + 1].to_broadcast([128, COLS]),
                    mybir.AluOpType.add,
                ).then_inc(s_add, 1)

        @block.scalar
        def _(scalar):
            for i in range(N_TILES):
                scalar.wait_ge(s_add, i + 1)
                scalar.dma_start(tile_dst(i), xts[i][:, :]).then_inc(s_store, 16)
            scalar.wait_ge(s_store, 16 * N_TILES)

    return nc


def kernel(x, context, gn_w=None, gn_b=None, Wq=None, bq=None, Wkv=None,
           bkv=None, Wout=None, bout=None, _trace=False):
    # gn_w/gn_b/Wq/bq and the k-half of Wkv/bkv are mathematically dead
    # (softmax over a length-1 axis is exactly 1), so they are unused.
    x = np.ascontiguousarray(np.asarray(x, dtype=np.float32))
    context = np.ascontiguousarray(np.asarray(context, dtype=np.float32))
    Wkv = np.asarray(Wkv, dtype=np.float32)
    bkv = np.asarray(bkv, dtype=np.float32)
    wkvT = np.ascontiguousarray(Wkv[C:2 * C].T)
    bkv_v = np.ascontiguousarray(bkv[C:2 * C].reshape(CC, 128).T)
    woT = np.ascontiguousarray(np.asarray(Wout, dtype=np.float32).T)
    bout_np = np.ascontiguousarray(
        np.asarray(bout, dtype=np.float32).reshape(CC, 128).T
    )

    if "nc" not in _cache:
        _cache["nc"] = _build_nc()
    nc = _cache["nc"]

    in_maps = []
    for c in range(N_CORES):
        xs = x[c * B_LOC:(c + 1) * B_LOC].reshape(ROWS, HW)
        in_maps.append({
            "xs": np.ascontiguousarray(xs),
            "ctxT": np.ascontiguousarray(context[c * B_LOC:(c + 1) * B_LOC].T),
            "wkvT": wkvT,
            "bkv_v": bkv_v,
            "woT": woT,
            "bout": bout_np,
        })

    res = run_bass_kernel_spmd(nc, in_maps, core_ids=list(range(N_CORES)),
                               trace=_trace)
    kernel.last_result = res
    out = np.concatenate(
        [r["out"].reshape(B_LOC, C, 64, 64) for r in res.results], axis=0
    )
    return out


# revision 10
# speedup vs baseline: 1.1763x; 1.1426x over previous
"""Trainium2 Bass kernel for nn_CrossAttentionBlock (raw Bass, no Tile).

Math note: the reference's attention has a length-1 key axis, so
softmax(attn, axis=-1) == 1.0 exactly and the attention output equals v
broadcast over the HW query axis.  The GroupNorm -> Wq -> q@k path is
therefore mathematically dead.  The exact output is

    out[b, c, h, w] = x[b, c, h, w] + y[b, c]
    y[b]            = Wout @ v[b] + bout
    v[b]            = Wkv[C:2C, :] @ context[b] + bkv[C:2C]

Sharding: pure data parallel over batch B=32 -> 4 batches per core on
8 cores; the small weights are replicated (passed pre-transposed so the
TensorEngine consumes them directly as matmul lhsT).  Per core the
kernel computes the tiny matmuls on the TensorEngine and streams the
16.8 MB x-shard through SBUF adding the per-(b,c) scalar — the kernel
is HBM-bandwidth-bound (~427 GB/s/core sustained on both DMA rings).

Raw engine programs with manual semaphores (no Tile/Bacc framework
barriers):
  sync   : x tile 0, the 5 small weight DMAs, x tiles 1-15 (HWDGE ring)
  tensor : 12 tiny matmuls (PSUM, one full bank per tile)
  vector : v/yb bias adds, then per-tile broadcast add (in place)
  scalar : per-tile store DMAs on the other HWDGE ring + final wait
All 16 x-tiles are SBUF-resident (no buffer reuse, no load gating).
"""

import numpy as np

import concourse.bass as bass
import concourse.mybir as mybir
from concourse.bass_utils import run_bass_kernel_spmd

N_CORES = 8
B = 32
C = 256
HW = 64 * 64
CTX = 512
B_LOC = B // N_CORES
ROWS = B_LOC * C                 # 1024
COLS = 2048                      # 1MB tiles [128, 2048]
N_TILES = (ROWS // 128) * (HW // COLS)   # 16
KC = CTX // 128                  # 4
CC = C // 128                    # 2
FP32 = mybir.dt.float32

_cache: dict = {}


def _build_nc() -> bass.Bass:
    nc = bass.Bass(target_bir_lowering=False)

    xs = nc.dram_tensor("xs", [ROWS, HW], FP32, kind="ExternalInput")
    ctxT = nc.dram_tensor("ctxT", [CTX, B_LOC], FP32, kind="ExternalInput")
    wkvT = nc.dram_tensor("wkvT", [CTX, C], FP32, kind="ExternalInput")
    # biases pre-striped on host to [128, CC] (partition p, chunk o)
    bkv_v = nc.dram_tensor("bkv_v", [128, CC], FP32, kind="ExternalInput")
    woT = nc.dram_tensor("woT", [C, C], FP32, kind="ExternalInput")
    bout = nc.dram_tensor("bout", [128, CC], FP32, kind="ExternalInput")
    out = nc.dram_tensor("out", [ROWS, HW], FP32, kind="ExternalOutput")

    def tile_src(idx):
        t, j = idx // 2, idx % 2
        return xs[t * 128:(t + 1) * 128, j * COLS:(j + 1) * COLS]

    def tile_dst(idx):
        t, j = idx // 2, idx % 2
        return out[t * 128:(t + 1) * 128, j * COLS:(j + 1) * COLS]

    def bias_col(idx):
        t = idx // 2
        return (t % CC) * B_LOC + t // CC   # column in yb [128, CC*B_LOC]

    xts = [nc.alloc_sbuf_tensor(f"xt{i}", [128, COLS], FP32) for i in range(N_TILES)]

    # one sem per load: with several DMAs in flight on one sem, the 16
    # per-SDMA-engine unit-increments can interleave across DMAs, so a
    # partial-progress wait (>= 16*(i+1)) would not imply tile i landed.
    # Dedicated sems make the per-tile wait exact; total-completion waits
    # (s_w >= 80, s_store >= 256) are safe on a shared sem.
    s_loads = [nc.alloc_semaphore(f"s_load{i}") for i in range(N_TILES)]

    with (
        nc.Block() as block,
        nc.semaphore("s_w") as s_w,
        nc.semaphore("s_mm") as s_mm,
        nc.semaphore("s_v") as s_v,
        nc.semaphore("s_add") as s_add,
        nc.semaphore("s_store") as s_store,
        nc.sbuf_tensor("wkvT_sb", [128, KC, C], FP32) as wkvT_sb,
        nc.sbuf_tensor("woT_sb", [128, CC, C], FP32) as woT_sb,
        nc.sbuf_tensor("ctxT_sb", [128, KC, B_LOC], FP32) as ctxT_sb,
        nc.sbuf_tensor("bkv_sb", [128, CC], FP32) as bkv_sb,
        nc.sbuf_tensor("bout_sb", [128, CC], FP32) as bout_sb,
        nc.sbuf_tensor("v_sb", [128, CC * B_LOC], FP32) as v_sb,
        nc.sbuf_tensor("yb", [128, CC * B_LOC], FP32) as yb,
        nc.psum_tensor("pv0", [128, 512], FP32) as pv0,
        nc.psum_tensor("pv1", [128, 512], FP32) as pv1,
        nc.psum_tensor("py0", [128, 512], FP32) as py0,
        nc.psum_tensor("py1", [128, 512], FP32) as py1,
    ):
        pv = [pv0, pv1]
        py = [py0, py1]

        @block.sync
        def _(sync):
            # first x tile first (its add gates the first store), then the
            # small weights, then the rest of the x tiles
            sync.dma_start(xts[0][:, :], tile_src(0)).then_inc(s_loads[0], 16)
            sync.dma_start(
                ctxT_sb[:, :, :], ctxT.rearrange("(o p) b -> p o b", p=128)
            ).then_inc(s_w, 16)
            sync.dma_start(
                wkvT_sb[:, :, :], wkvT.rearrange("(o p) c -> p o c", p=128)
            ).then_inc(s_w, 16)
            sync.dma_start(
                woT_sb[:, :, :], woT.rearrange("(s p) o -> p s o", p=128)
            ).then_inc(s_w, 16)
            sync.dma_start(bkv_sb[:, :], bkv_v[:, :]).then_inc(s_w, 16)
            sync.dma_start(bout_sb[:, :], bout[:, :]).then_inc(s_w, 16)
            for i in range(1, N_TILES):
                sync.dma_start(xts[i][:, :], tile_src(i)).then_inc(s_loads[i], 16)

        @block.tensor
        def _(tensor):
            tensor.wait_ge(s_w, 80)
            # v[c, b] = Wkv_v @ ctx^T  (2 c-chunks x 4 k-chunks)
            for cc in range(CC):
                for kc in range(KC):
                    nc.tensor.matmul(
                        pv[cc][:, :B_LOC],
                        wkvT_sb[:, kc, cc * 128:cc * 128 + 128],
                        ctxT_sb[:, kc, :],
                        start=(kc == 0),
                        stop=(kc == KC - 1),
                    )
                nc.tensor.drain().then_inc(s_mm, 1)
            # y[o, b] = Wout @ v  (needs v_sb from vector)
            tensor.wait_ge(s_v, 2)
            for oc in range(CC):
                for cc in range(CC):
                    nc.tensor.matmul(
                        py[oc][:, :B_LOC],
                        woT_sb[:, cc, oc * 128:oc * 128 + 128],
                        v_sb[:, cc * B_LOC:(cc + 1) * B_LOC],
                        start=(cc == 0),
                        stop=(cc == CC - 1),
                    )
                nc.tensor.drain().then_inc(s_mm, 1)

        @block.vector
        def _(vector):
            for cc in range(CC):
                vector.wait_ge(s_mm, cc + 1)
                nc.vector.tensor_tensor(
                    v_sb[:, cc * B_LOC:(cc + 1) * B_LOC],
                    pv[cc][:, :B_LOC],
                    bkv_sb[:, cc:cc + 1].to_broadcast([128, B_LOC]),
                    mybir.AluOpType.add,
                ).then_inc(s_v, 1)
            for oc in range(CC):
                vector.wait_ge(s_mm, CC + oc + 1)
                nc.vector.tensor_tensor(
                    yb[:, oc * B_LOC:(oc + 1) * B_LOC],
                    py[oc][:, :B_LOC],
                    bout_sb[:, oc:oc + 1].to_broadcast([128, B_LOC]),
                    mybir.AluOpType.add,
                )
            # drain the DVE pipeline: the tile adds read yb written above
            # on the same engine (deep pipeline, in-order but uncommitted)
            nc.vector.drain()
            for i in range(N_TILES):
                vector.wait_ge(s_loads[i], 16)
                c = bias_col(i)
                nc.vector.tensor_tensor(
                    xts[i][:, :],
                    xts[i][:, :],
                    yb[:, c:c + 1].to_broadcast([128, COLS]),
                    mybir.AluOpType.add,
                ).then_inc(s_add, 1)

        @block.scalar
        def _(scalar):
            for i in range(N_TILES):
                scalar.wait_ge(s_add, i + 1)
                scalar.dma_start(tile_dst(i), xts[i][:, :]).then_inc(s_store, 16)
            scalar.wait_ge(s_store, 16 * N_TILES)

    return nc


def kernel(x, context, gn_w=None, gn_b=None, Wq=None, bq=None, Wkv=None,
           bkv=None, Wout=None, bout=None, _trace=False):
    # gn_w/gn_b/Wq/bq and the k-half of Wkv/bkv are mathematically dead
    # (softmax over a length-1 axis is exactly 1), so they are unused.
    x = np.ascontiguousarray(np.asarray(x, dtype=np.float32))
    context = np.ascontiguousarray(np.asarray(context, dtype=np.float32))
    Wkv = np.asarray(Wkv, dtype=np.float32)
    bkv = np.asarray(bkv, dtype=np.float32)
    wkvT = np.ascontiguousarray(Wkv[C:2 * C].T)
    bkv_v = np.ascontiguousarray(bkv[C:2 * C].reshape(CC, 128).T)
    woT = np.ascontiguousarray(np.asarray(Wout, dtype=np.float32).T)
    bout_np = np.ascontiguousarray(
        np.asarray(bout, dtype=np.float32).reshape(CC, 128).T
    )

    if "nc" not in _cache:
        _cache["nc"] = _build_nc()
    nc = _cache["nc"]

    in_maps = []
    for c in range(N_CORES):
        xs = x[c * B_LOC:(c + 1) * B_LOC].reshape(ROWS, HW)
        in_maps.append({
            "xs": np.ascontiguousarray(xs),
            "ctxT": np.ascontiguousarray(context[c * B_LOC:(c + 1) * B_LOC].T),
            "wkvT": wkvT,
            "bkv_v": bkv_v,
            "woT": woT,
            "bout": bout_np,
        })

    res = run_bass_kernel_spmd(nc, in_maps, core_ids=list(range(N_CORES)),
                               trace=_trace)
    kernel.last_result = res
    out = np.concatenate(
        [r["out"].reshape(B_LOC, C, 64, 64) for r in res.results], axis=0
    )
    return out
